# revision 14
# baseline (speedup 1.0000x reference)
"""Trainium2 Bass kernel for nn_GaussianLayer (segment_reduce).

Computes ll[b, r, k] = -0.5 * sum_d((x[b, regions[r,d]] - means[r,k,d]) / scales[r,k,d])^2
                       - sum_d log(scales[r,k,d]) - 0.5 * D * log(2*pi)

Strategy (data-parallel over batch across 8 cores, 512 rows each):
  Host folds the small [R,K,D] params into matmul weights:
      ll = Xsq @ Wsq + Xraw @ Wraw + const
  where Xraw[b, (r,d)] = x[b, regions[r,d]] (the gather), Xsq = Xraw^2,
  Wsq = -0.5/scales^2, Wraw = means/scales^2 (block-diagonal per region),
  const[r,k] = -0.5*sum_d(means^2/scales^2) - sum_d log(scales) - 0.5*D*LOG2PI.

  Device, per core (output computed TRANSPOSED: outT[rk, b]):
    phase 1: DMA x f32 -> PE-transpose (f32) -> DVE copy to bf16 ->
        xT[1024 f, 512 b] bf16 -> HBM scratch.  PE warm-up matmuls keep the
        tensor engine clock ramped.
    phase 2: ONE 1024-idx gpsimd.dma_gather (descriptors prepared early via
        prepare_only, fired with trigger_dma) pulls all gathered feature rows
        into SBUF gt [128, 8 pairs, 512 b].
    phase 3: per rk-block of 128: matmul(lhsT=W[:,blk], rhs=gt pair) +
        matmul(lhsT=Wsq[:,blk], rhs=sq pair) accumulate in PSUM [128 rk, 512 b];
        Scalar ACT Identity drains PSUM -> SBUF bf16 with per-partition bias
        = const (fused const-add).  Stores are bf16 (host casts back to f32).
"""

import os
import sys

for _p in ("/opt/trn_rl_repo", "/root/.axon_site/_ro/trn_rl_repo"):
    if os.path.isdir(_p) and _p not in sys.path:
        sys.path.insert(0, _p)

import numpy as np
import ml_dtypes

import concourse.bass as bass
import concourse.tile as tile
from concourse import bacc, library_config, mybir
from concourse.bass_utils import run_bass_kernel_spmd

LOG_2PI = 1.8378770664093453
B, F = 4096, 1024
R, K, D = 64, 32, 16
NCORES = 8
BL = B // NCORES      # 512 batch rows per core
NT = BL // 128        # 4 batch tiles per core
RKCOLS = R * K        # 2048 output columns
NPAIR = 8             # pair = 8 regions = 128 gathered rows
NBLK = 16             # rk blocks of 128 (2 per pair)

# PE warm-up matmul counts (keep the HAM clock ramped through idle windows)
N_PRE = 8             # before tile-0 transposes (covers x DMA window)
N_MID = 3             # between batch tiles
N_PRE3 = 10           # covering the gather window

_module_cache = {}


def _build_module():
    if "nc" in _module_cache:
        return _module_cache["nc"]

    nc = bacc.Bacc(
        trn_type="TRN2",
        target_bir_lowering=False,
        debug=False,
        enable_asserts=False,
        num_swdge_queues=2,
    )
    bf16 = mybir.dt.bfloat16
    f32 = mybir.dt.float32
    i16 = mybir.dt.int16

    x_d = nc.dram_tensor("x", [BL, F], f32, kind="ExternalInput").ap()
    wraw_d = nc.dram_tensor("wraw", [128, RKCOLS], bf16, kind="ExternalInput").ap()
    wsq_d = nc.dram_tensor("wsq", [128, RKCOLS], bf16, kind="ExternalInput").ap()
    cstt_d = nc.dram_tensor("cstt", [128, NBLK], f32, kind="ExternalInput").ap()
    idx_d = nc.dram_tensor("idx", [128, F // 16], i16, kind="ExternalInput").ap()
    id_d = nc.dram_tensor("ident", [128, 128], f32, kind="ExternalInput").ap()
    out_d = nc.dram_tensor("out", [NBLK, 128, BL], bf16, kind="ExternalOutput").ap()

    with tile.TileContext(nc) as tc:
        with (
            tc.tile_pool(name="persist", bufs=1) as persist,
            tc.tile_pool(name="dram", bufs=1, space="DRAM") as drampool,
            tc.tile_pool(name="xin", bufs=3) as xpool,
            tc.tile_pool(name="trp", bufs=2, space="PSUM") as trpool,
            tc.tile_pool(name="wrm", bufs=1, space="PSUM") as warmpool,
            tc.tile_pool(name="xts", bufs=2) as xtspool,
            tc.tile_pool(name="gt", bufs=1) as gtpool,
            tc.tile_pool(name="sq", bufs=1) as sqpool,
            tc.tile_pool(name="po", bufs=5, space="PSUM") as popool,
            tc.tile_pool(name="osb", bufs=2) as opool,
        ):
            # warm-up source with no DMA dependency (memset on gpsimd)
            warmsrc = persist.tile([128, 512], bf16)
            nc.gpsimd.memset(warmsrc[:], 0.5)
            nc.gpsimd.load_library(library_config.mlp)

            # params: idx first on sync (gather prep depends on it)
            idx = persist.tile([128, F // 16], i16)
            nc.sync.dma_start(idx[:], idx_d)
            ident = persist.tile([128, 128], f32)
            nc.scalar.dma_start(ident[:], id_d)
            w_raw = persist.tile([128, RKCOLS], bf16)
            nc.scalar.dma_start(w_raw[:], wraw_d)
            w_sq = persist.tile([128, RKCOLS], bf16)
            nc.scalar.dma_start(w_sq[:], wsq_d)
            cstt = persist.tile([128, NBLK], f32)
            nc.scalar.dma_start(cstt[:], cstt_d)

            # x tile loads all issued up front on the sync DGE
            xtiles = []
            for bt in range(NT):
                xf = xpool.tile([128, F], f32, tag=f"x{bt}")
                nc.sync.dma_start(xf[:], x_d[bt * 128:(bt + 1) * 128, :])
                xtiles.append(xf)

            # HBM scratch holding xT (feature-major, bf16): row f = 512 vals
            xt_dram = drampool.tile([F, BL], bf16)
            xt_wview = xt_dram[:].rearrange("(c p) b -> p c b", p=128)

            # gathered tile: [128 p, pair, 512 b]; gathered row i lands at
            # partition i%128, chunk i//128 -> chunk j = pair j's 128 rows
            gt = gtpool.tile([128, NPAIR * BL], bf16)
            gsems = [nc.alloc_semaphore(f"gsem{q}") for q in range(2)]
            gt_view = gt[:].rearrange("p (c b) -> p c b", c=NPAIR)
            xt_rows = xt_dram[:].rearrange("(a f) b -> a f b", a=1)[0]

            # trigger the Scalar ACT table load early (off the critical path)
            actwarm = persist.tile([128, 1], bf16)
            nc.scalar.activation(
                actwarm[:], cstt[:, 0:1],
                mybir.ActivationFunctionType.Identity,
                bias=cstt[:, 0:1],
            )

            warm = warmpool.tile([128, 512], f32)

            def warm_mm(n):
                for _ in range(n):
                    nc.tensor.matmul(
                        warm[:], warmsrc[:, 0:128], warmsrc[:],
                        start=True, stop=True,
                    )

            # ---- phase 1: transpose x into xT (HBM), f32 on the PE ----
            warm_mm(N_PRE)
            for bt in range(NT):
                xf = xtiles[bt]
                xts = xtspool.tile([128, F], bf16)  # [128, 8 chunks, 128 b]
                for half in range(2):
                    pt = trpool.tile([128, 512], f32)
                    for j in range(4):
                        c = 4 * half + j
                        nc.tensor.transpose(
                            pt[:, j * 128:(j + 1) * 128],
                            xf[:, c * 128:(c + 1) * 128],
                            ident[:],
                        )
                    nc.vector.tensor_copy(
                        xts[:, half * 512:(half + 1) * 512], pt[:]
                    )
                nc.sync.dma_start(
                    xt_wview[:, :, bt * 128:(bt + 1) * 128],
                    xts[:].rearrange("p (c b) -> p c b", c=8),
                )
                if bt < NT - 1:
                    warm_mm(N_MID)

            # ---- phase 2: prepare + fire the gathers.  Emitted AFTER the xT
            # writes so Tile sees the RAW edges and defers them onto the
            # triggers; desc-gen itself still runs early on gpsimd (its only
            # sync dep is idx) ----
            for q in range(2):
                nc.gpsimd.dma_gather(
                    out_ap=gt_view[:, 4 * q:4 * q + 4],
                    in_ap=xt_rows,
                    idxs_ap=idx[:, 32 * q:32 * q + 32],
                    num_idxs=F // 2,
                    num_idxs_reg=F // 2,
                    elem_size=BL,
                    prepare_only=True,
                    sem=gsems[q],
                    queue_num=q,
                )
            nc.gpsimd.trigger_dma(count=None, queue_num=0)
            nc.gpsimd.trigger_dma(count=None, queue_num=1)
            warm_mm(N_PRE3)
            # explicit completion waits (prep path does not auto-sync readers)
            for q in range(2):
                nc.tensor.wait_ge(gsems[q], 16)
                nc.vector.wait_ge(gsems[q], 16)

            # ---- phase 3: outT block matmuls + fused const-add drain ----
            sq = sqpool.tile([128, NPAIR * BL], bf16)
            osb = None
            for j in range(NPAIR):
                js = slice(j * BL, (j + 1) * BL)
                nc.vector.tensor_mul(sq[:, js], gt[:, js], gt[:, js])
                for h in range(2):
                    g = 2 * j + h
                    slot = g % 4
                    if slot == 0:
                        osb = opool.tile([128, 4 * BL], bf16)
                    po = popool.tile([128, BL], f32)
                    nc.tensor.matmul(
                        po[:], w_raw[:, g * 128:(g + 1) * 128], gt[:, js],
                        start=True, stop=False,
                    )
                    nc.tensor.matmul(
                        po[:], w_sq[:, g * 128:(g + 1) * 128], sq[:, js],
                        start=False, stop=True,
                    )
                    if h == 0:
                        nc.scalar.activation(
                            osb[:, slot * BL:(slot + 1) * BL], po[:],
                            mybir.ActivationFunctionType.Identity,
                            bias=cstt[:, g:g + 1],
                        )
                    else:
                        nc.vector.tensor_scalar_add(
                            osb[:, slot * BL:(slot + 1) * BL], po[:],
                            cstt[:, g:g + 1],
                        )
                    if slot == 3:
                        q = g // 4
                        eng = nc.sync if (q % 2 == 0) else nc.scalar
                        eng.dma_start(
                            out_d[4 * q:4 * q + 4].rearrange("c p b -> p c b"),
                            osb[:].rearrange("p (c b) -> p c b", c=4),
                        )

    nc.compile()
    _module_cache["nc"] = nc
    return nc


def _prep_params(regions, means, scales):
    """Host folding of the small [R,K,D] params into matmul weights."""
    regions = np.asarray(regions).astype(np.int64)
    means = np.asarray(means, dtype=np.float64)
    scales = np.asarray(scales, dtype=np.float64)

    inv2 = 1.0 / scales**2                                   # [R,K,D]
    wsq_c = -0.5 * inv2                                      # coeff of x^2
    wraw_c = means * inv2                                    # coeff of x
    const = (
        -0.5 * np.sum(means**2 * inv2, axis=-1)
        - np.sum(np.log(scales), axis=-1)
        - 0.5 * D * LOG_2PI
    )                                                        # [R,K]

    # Block-diagonal weight tiles: pair p covers regions 8p..8p+7.
    # Row 16j+d (region-local j in 0..7), col 32j+k within the pair's 256.
    wraw = np.zeros((128, RKCOLS), np.float32)
    wsq = np.zeros((128, RKCOLS), np.float32)
    for p in range(NPAIR):
        for j in range(8):
            r = 8 * p + j
            rows = slice(16 * j, 16 * j + 16)
            cols = slice(256 * p + 32 * j, 256 * p + 32 * j + 32)
            wraw[rows, cols] = wraw_c[r].T.astype(np.float32)   # [D, K]
            wsq[rows, cols] = wsq_c[r].T.astype(np.float32)
    wraw = wraw.astype(ml_dtypes.bfloat16)
    wsq = wsq.astype(ml_dtypes.bfloat16)

    # per-partition const bias, one column per rk-block of 128:
    # flat col 32r + k -> block g = col // 128, row = col % 128
    cstt = (
        const.reshape(-1).astype(np.float32).reshape(NBLK, 128).T.copy()
    )                                                        # [128, 16]

    # dma_gather index layout: index j lives at [j % 16, j // 16],
    # replicated across the eight 16-partition groups.
    perm = regions.reshape(-1).astype(np.int16)              # [1024]
    idx16 = perm.reshape(F // 16, 16).T                      # [16, 64]
    idx = np.tile(idx16, (8, 1)).copy()                      # [128, 64]

    ident = np.eye(128, dtype=np.float32)
    return wraw, wsq, cstt, idx, ident


def _run(inputs, trace=False, **kwargs):
    x = np.ascontiguousarray(np.asarray(inputs["x"], dtype=np.float32))
    assert x.shape == (B, F), x.shape
    wraw, wsq, cstt, idx, ident = _prep_params(
        inputs["regions"], inputs["means"], inputs["scales"]
    )

    nc = _build_module()
    in_maps = []
    for c in range(NCORES):
        in_maps.append({
            "x": np.ascontiguousarray(x[c * BL:(c + 1) * BL]),
            "wraw": wraw,
            "wsq": wsq,
            "cstt": cstt,
            "idx": idx,
            "ident": ident,
        })
    res = run_bass_kernel_spmd(
        nc, in_maps, core_ids=list(range(NCORES)), trace=trace, **kwargs
    )
    # reassemble: per core outT [16, 128, 512] bf16 -> [2048 rk, 512 b]
    parts = []
    for c in range(NCORES):
        ot = np.asarray(res.results[c]["out"]).reshape(RKCOLS, BL)
        parts.append(ot.T)                                   # [512 b, 2048 rk]
    out = np.concatenate(parts, axis=0).astype(np.float32).reshape(B, R, K)
    return out, res


def kernel(**inputs):
    out, _ = _run(inputs, trace=False)
    return out


# revision 20
# speedup vs baseline: 1.0320x; 1.0320x over previous
"""Trainium2 Bass kernel for nn_GaussianLayer (segment_reduce).

Computes ll[b, r, k] = -0.5 * sum_d((x[b, regions[r,d]] - means[r,k,d]) / scales[r,k,d])^2
                       - sum_d log(scales[r,k,d]) - 0.5 * D * log(2*pi)

Strategy (data-parallel over batch across 8 cores, 512 rows each):
  Host folds the small [R,K,D] params into matmul weights:
      ll = Xsq @ Wsq + Xraw @ Wraw + const
  where Xraw[b, (r,d)] = x[b, regions[r,d]] (the gather), Xsq = Xraw^2,
  Wsq = -0.5/scales^2, Wraw = means/scales^2 (block-diagonal per region),
  const[r,k] = -0.5*sum_d(means^2/scales^2) - sum_d log(scales) - 0.5*D*LOG2PI.

  Device, per core (output computed TRANSPOSED: outT[rk, b]):
    phase 1: DMA x f32 -> PE-transpose (f32) -> DVE copy to bf16 ->
        xT[1024 f, 512 b] bf16 -> HBM scratch.  PE warm-up matmuls keep the
        tensor engine clock ramped.
    phase 2: ONE 1024-idx gpsimd.dma_gather (descriptors prepared early via
        prepare_only, fired with trigger_dma) pulls all gathered feature rows
        into SBUF gt [128, 8 pairs, 512 b].
    phase 3: per rk-block of 128: matmul(lhsT=W[:,blk], rhs=gt pair) +
        matmul(lhsT=Wsq[:,blk], rhs=sq pair) accumulate in PSUM [128 rk, 512 b];
        Scalar ACT Identity drains PSUM -> SBUF bf16 with per-partition bias
        = const (fused const-add).  Stores are bf16 (host casts back to f32).
"""

import os
import sys

for _p in ("/opt/trn_rl_repo", "/root/.axon_site/_ro/trn_rl_repo"):
    if os.path.isdir(_p) and _p not in sys.path:
        sys.path.insert(0, _p)

import numpy as np
import ml_dtypes

import concourse.bass as bass
import concourse.tile as tile
from concourse import bacc, library_config, mybir
from concourse.bass_utils import run_bass_kernel_spmd

LOG_2PI = 1.8378770664093453
B, F = 4096, 1024
R, K, D = 64, 32, 16
NCORES = 8
BL = B // NCORES      # 512 batch rows per core
NT = BL // 128        # 4 batch tiles per core
RKCOLS = R * K        # 2048 output columns
NPAIR = 8             # pair = 8 regions = 128 gathered rows
NBLK = 16             # rk blocks of 128 (2 per pair)

# PE warm-up matmul counts (keep the HAM clock ramped through idle windows)
N_PRE = 8             # before tile-0 transposes (covers x DMA window)
N_MID = 3             # between batch tiles
N_PRE3 = 10           # covering the gather window

_module_cache = {}


def _build_module():
    if "nc" in _module_cache:
        return _module_cache["nc"]

    nc = bacc.Bacc(
        trn_type="TRN2",
        target_bir_lowering=False,
        debug=False,
        enable_asserts=False,
        num_swdge_queues=2,
    )
    bf16 = mybir.dt.bfloat16
    f32 = mybir.dt.float32
    i16 = mybir.dt.int16

    x_d = nc.dram_tensor("x", [BL, F], f32, kind="ExternalInput").ap()
    wraw_d = nc.dram_tensor("wraw", [128, RKCOLS], bf16, kind="ExternalInput").ap()
    wsq_d = nc.dram_tensor("wsq", [128, RKCOLS], bf16, kind="ExternalInput").ap()
    cstt_d = nc.dram_tensor("cstt", [128, NBLK], f32, kind="ExternalInput").ap()
    idx_d = nc.dram_tensor("idx", [128, F // 16], i16, kind="ExternalInput").ap()
    id_d = nc.dram_tensor("ident", [128, 128], bf16, kind="ExternalInput").ap()
    out_d = nc.dram_tensor("out", [NBLK, 128, BL], bf16, kind="ExternalOutput").ap()

    with tile.TileContext(nc) as tc:
        with (
            tc.tile_pool(name="persist", bufs=1) as persist,
            tc.tile_pool(name="dram", bufs=1, space="DRAM") as drampool,
            tc.tile_pool(name="xin", bufs=3) as xpool,
            tc.tile_pool(name="xgb", bufs=2) as xgbpool,
            tc.tile_pool(name="trp", bufs=2, space="PSUM") as trpool,
            tc.tile_pool(name="wrm", bufs=1, space="PSUM") as warmpool,
            tc.tile_pool(name="xts", bufs=2) as xtspool,
            tc.tile_pool(name="gt", bufs=1) as gtpool,
            tc.tile_pool(name="sq", bufs=1) as sqpool,
            tc.tile_pool(name="po", bufs=5, space="PSUM") as popool,
            tc.tile_pool(name="osb", bufs=2) as opool,
        ):
            # warm-up source with no DMA dependency (memset on gpsimd)
            warmsrc = persist.tile([128, 512], bf16)
            nc.gpsimd.memset(warmsrc[:], 0.5)
            nc.gpsimd.load_library(library_config.mlp)

            # params: idx first on sync (gather prep depends on it); small
            # tiles before the big weight tiles on scalar
            idx = persist.tile([128, F // 16], i16)
            nc.sync.dma_start(idx[:], idx_d)
            ident = persist.tile([128, 128], bf16)
            nc.scalar.dma_start(ident[:], id_d)
            cstt = persist.tile([128, NBLK], f32)
            nc.scalar.dma_start(cstt[:], cstt_d)
            w_raw = persist.tile([128, RKCOLS], bf16)
            nc.scalar.dma_start(w_raw[:], wraw_d)
            w_sq = persist.tile([128, RKCOLS], bf16)
            nc.scalar.dma_start(w_sq[:], wsq_d)

            # x tile loads all issued up front on the sync DGE
            xtiles = []
            for bt in range(NT):
                xf = xpool.tile([128, F], f32, tag=f"x{bt}")
                nc.sync.dma_start(xf[:], x_d[bt * 128:(bt + 1) * 128, :])
                xtiles.append(xf)

            # HBM scratch holding xT (feature-major, bf16): row f = 512 vals
            xt_dram = drampool.tile([F, BL], bf16)
            xt_wview = xt_dram[:].rearrange("(c p) b -> p c b", p=128)

            # gathered tile: [128 p, pair, 512 b]; gathered row i lands at
            # partition i%128, chunk i//128 -> chunk j = pair j's 128 rows
            gt = gtpool.tile([128, NPAIR * BL], bf16)
            gsems = [nc.alloc_semaphore(f"gsem{q}") for q in range(2)]
            gt_view = gt[:].rearrange("p (c b) -> p c b", c=NPAIR)
            xt_rows = xt_dram[:].rearrange("(a f) b -> a f b", a=1)[0]

            # trigger the Scalar ACT table load early (off the critical path)
            actwarm = persist.tile([128, 1], bf16)
            nc.scalar.activation(
                actwarm[:], cstt[:, 0:1],
                mybir.ActivationFunctionType.Identity,
                bias=cstt[:, 0:1],
            )

            warm = warmpool.tile([128, 512], f32)

            def warm_mm(n):
                for _ in range(n):
                    nc.tensor.matmul(
                        warm[:], warmsrc[:, 0:128], warmsrc[:],
                        start=True, stop=True,
                    )

            # ---- phase 1: cast on Scalar, transpose bf16 on PE ----
            warm_mm(N_PRE)
            for bt in range(NT):
                xf = xtiles[bt]
                xgb = xgbpool.tile([128, F], bf16)
                nc.scalar.copy(xgb[:], xf[:])
                xts = xtspool.tile([128, F], bf16)  # [128, 8 chunks, 128 b]
                for half in range(2):
                    pt = trpool.tile([128, 512], bf16)
                    for j in range(4):
                        c = 4 * half + j
                        nc.tensor.transpose(
                            pt[:, j * 128:(j + 1) * 128],
                            xgb[:, c * 128:(c + 1) * 128],
                            ident[:],
                        )
                    nc.vector.tensor_copy(
                        xts[:, half * 512:(half + 1) * 512], pt[:]
                    )
                nc.sync.dma_start(
                    xt_wview[:, :, bt * 128:(bt + 1) * 128],
                    xts[:].rearrange("p (c b) -> p c b", c=8),
                )
                if bt < NT - 1:
                    warm_mm(N_MID)

            # ---- phase 2: prepare + fire the gather.  Emitted AFTER the xT
            # writes so Tile sees the RAW edges and defers them onto the
            # trigger; desc-gen itself runs early on gpsimd (its only sync
            # dep is idx) ----
            nc.gpsimd.dma_gather(
                out_ap=gt_view,
                in_ap=xt_rows,
                idxs_ap=idx[:],
                num_idxs=F,
                num_idxs_reg=F,
                elem_size=BL,
                prepare_only=True,
                sem=gsems[0],
                queue_num=0,
            )
            nc.gpsimd.trigger_dma(count=None, queue_num=0)
            warm_mm(N_PRE3)
            # explicit completion waits (prep path does not auto-sync readers)
            nc.tensor.wait_ge(gsems[0], 16)
            nc.vector.wait_ge(gsems[0], 16)

            # ---- phase 3: outT block matmuls + fused const-add drain ----
            sq = sqpool.tile([128, NPAIR * BL], bf16)
            osb = None
            for j in range(NPAIR):
                js = slice(j * BL, (j + 1) * BL)
                nc.vector.tensor_mul(sq[:, js], gt[:, js], gt[:, js])
                for h in range(2):
                    g = 2 * j + h
                    slot = g % 4
                    if slot == 0:
                        osb = opool.tile([128, 4 * BL], bf16)
                    po = popool.tile([128, BL], f32)
                    nc.tensor.matmul(
                        po[:], w_raw[:, g * 128:(g + 1) * 128], gt[:, js],
                        start=True, stop=False,
                    )
                    nc.tensor.matmul(
                        po[:], w_sq[:, g * 128:(g + 1) * 128], sq[:, js],
                        start=False, stop=True,
                    )
                    if h == 0:
                        nc.scalar.activation(
                            osb[:, slot * BL:(slot + 1) * BL], po[:],
                            mybir.ActivationFunctionType.Identity,
                            bias=cstt[:, g:g + 1],
                        )
                    else:
                        nc.vector.tensor_scalar_add(
                            osb[:, slot * BL:(slot + 1) * BL], po[:],
                            cstt[:, g:g + 1],
                        )
                    if slot == 3:
                        q = g // 4
                        eng = nc.sync
                        eng.dma_start(
                            out_d[4 * q:4 * q + 4].rearrange("c p b -> p c b"),
                            osb[:].rearrange("p (c b) -> p c b", c=4),
                        )

    nc.compile()
    _module_cache["nc"] = nc
    return nc


def _prep_params(regions, means, scales):
    """Host folding of the small [R,K,D] params into matmul weights."""
    regions = np.asarray(regions).astype(np.int64)
    means = np.asarray(means, dtype=np.float64)
    scales = np.asarray(scales, dtype=np.float64)

    inv2 = 1.0 / scales**2                                   # [R,K,D]
    wsq_c = -0.5 * inv2                                      # coeff of x^2
    wraw_c = means * inv2                                    # coeff of x
    const = (
        -0.5 * np.sum(means**2 * inv2, axis=-1)
        - np.sum(np.log(scales), axis=-1)
        - 0.5 * D * LOG_2PI
    )                                                        # [R,K]

    # Block-diagonal weight tiles: pair p covers regions 8p..8p+7.
    # Row 16j+d (region-local j in 0..7), col 32j+k within the pair's 256.
    wraw = np.zeros((128, RKCOLS), np.float32)
    wsq = np.zeros((128, RKCOLS), np.float32)
    for p in range(NPAIR):
        for j in range(8):
            r = 8 * p + j
            rows = slice(16 * j, 16 * j + 16)
            cols = slice(256 * p + 32 * j, 256 * p + 32 * j + 32)
            wraw[rows, cols] = wraw_c[r].T.astype(np.float32)   # [D, K]
            wsq[rows, cols] = wsq_c[r].T.astype(np.float32)
    wraw = wraw.astype(ml_dtypes.bfloat16)
    wsq = wsq.astype(ml_dtypes.bfloat16)

    # per-partition const bias, one column per rk-block of 128:
    # flat col 32r + k -> block g = col // 128, row = col % 128
    cstt = (
        const.reshape(-1).astype(np.float32).reshape(NBLK, 128).T.copy()
    )                                                        # [128, 16]

    # dma_gather index layout: index j lives at [j % 16, j // 16],
    # replicated across the eight 16-partition groups.
    perm = regions.reshape(-1).astype(np.int16)              # [1024]
    idx16 = perm.reshape(F // 16, 16).T                      # [16, 64]
    idx = np.tile(idx16, (8, 1)).copy()                      # [128, 64]

    ident = np.eye(128, dtype=ml_dtypes.bfloat16)
    return wraw, wsq, cstt, idx, ident


def _run(inputs, trace=False, **kwargs):
    x = np.ascontiguousarray(np.asarray(inputs["x"], dtype=np.float32))
    assert x.shape == (B, F), x.shape
    wraw, wsq, cstt, idx, ident = _prep_params(
        inputs["regions"], inputs["means"], inputs["scales"]
    )

    nc = _build_module()
    in_maps = []
    for c in range(NCORES):
        in_maps.append({
            "x": np.ascontiguousarray(x[c * BL:(c + 1) * BL]),
            "wraw": wraw,
            "wsq": wsq,
            "cstt": cstt,
            "idx": idx,
            "ident": ident,
        })
    res = run_bass_kernel_spmd(
        nc, in_maps, core_ids=list(range(NCORES)), trace=trace, **kwargs
    )
    # reassemble: per core outT [16, 128, 512] bf16 -> [2048 rk, 512 b]
    parts = []
    for c in range(NCORES):
        ot = np.asarray(res.results[c]["out"]).reshape(RKCOLS, BL)
        parts.append(ot.T)                                   # [512 b, 2048 rk]
    out = np.concatenate(parts, axis=0).astype(np.float32).reshape(B, R, K)
    return out, res


def kernel(**inputs):
    out, _ = _run(inputs, trace=False)
    return out


# revision 24
# speedup vs baseline: 1.1346x; 1.0994x over previous
"""Trainium2 Bass kernel for nn_GaussianLayer (segment_reduce).

Computes ll[b, r, k] = -0.5 * sum_d((x[b, regions[r,d]] - means[r,k,d]) / scales[r,k,d])^2
                       - sum_d log(scales[r,k,d]) - 0.5 * D * log(2*pi)

Strategy (data-parallel over batch across 8 cores, 512 rows each):
  Host folds the small [R,K,D] params into matmul weights:
      ll = Xsq @ Wsq + Xraw @ Wraw + const
  where Xraw[b, (r,d)] = x[b, regions[r,d]] (the gather), Xsq = Xraw^2,
  Wsq = -0.5/scales^2, Wraw = means/scales^2 (block-diagonal per region),
  const[r,k] = -0.5*sum_d(means^2/scales^2) - sum_d log(scales) - 0.5*D*LOG2PI.

  Device, per core (output computed TRANSPOSED: outT[rk, b]):
    phase 1: DMA x f32 -> PE-transpose (f32) -> DVE copy to bf16 ->
        xT[1024 f, 512 b] bf16 -> HBM scratch.  PE warm-up matmuls keep the
        tensor engine clock ramped.
    phase 2: ONE 1024-idx gpsimd.dma_gather (descriptors prepared early via
        prepare_only, fired with trigger_dma) pulls all gathered feature rows
        into SBUF gt [128, 8 pairs, 512 b].
    phase 3: per rk-block of 128: matmul(lhsT=W[:,blk], rhs=gt pair) +
        matmul(lhsT=Wsq[:,blk], rhs=sq pair) accumulate in PSUM [128 rk, 512 b];
        Scalar ACT Identity drains PSUM -> SBUF bf16 with per-partition bias
        = const (fused const-add).  Stores are bf16 (host casts back to f32).
"""

import os
import sys

for _p in ("/opt/trn_rl_repo", "/root/.axon_site/_ro/trn_rl_repo"):
    if os.path.isdir(_p) and _p not in sys.path:
        sys.path.insert(0, _p)

import numpy as np
import ml_dtypes

import concourse.bass as bass
import concourse.tile as tile
from concourse import bacc, library_config, mybir
from concourse.bass_utils import run_bass_kernel_spmd

LOG_2PI = 1.8378770664093453
B, F = 4096, 1024
R, K, D = 64, 32, 16
NCORES = 8
BL = B // NCORES      # 512 batch rows per core
NT = BL // 128        # 4 batch tiles per core
RKCOLS = R * K        # 2048 output columns
NPAIR = 8             # pair = 8 regions = 128 gathered rows
NBLK = 16             # rk blocks of 128 (2 per pair)

# PE warm-up matmul counts (keep the HAM clock ramped through idle windows)
N_PRE = 8             # before tile-0 transposes (covers x DMA window)
N_MID = 3             # between batch tiles
N_PRE3 = 10           # covering the gather window

_module_cache = {}


def _build_module():
    if "nc" in _module_cache:
        return _module_cache["nc"]

    nc = bacc.Bacc(
        trn_type="TRN2",
        target_bir_lowering=False,
        debug=False,
        enable_asserts=False,
        num_swdge_queues=2,
    )
    bf16 = mybir.dt.bfloat16
    f32 = mybir.dt.float32
    i16 = mybir.dt.int16

    x_d = nc.dram_tensor("x", [BL, F], f32, kind="ExternalInput").ap()
    wraw_d = nc.dram_tensor("wraw", [128, RKCOLS], bf16, kind="ExternalInput").ap()
    wsq_d = nc.dram_tensor("wsq", [128, RKCOLS], bf16, kind="ExternalInput").ap()
    cstt_d = nc.dram_tensor("cstt", [128, NBLK], f32, kind="ExternalInput").ap()
    idx_d = nc.dram_tensor("idx", [128, F // 16], i16, kind="ExternalInput").ap()
    id_d = nc.dram_tensor("ident", [128, 128], bf16, kind="ExternalInput").ap()
    out_d = nc.dram_tensor("out", [NBLK, 128, BL], bf16, kind="ExternalOutput").ap()

    with tile.TileContext(nc) as tc:
        with (
            tc.tile_pool(name="persist", bufs=1) as persist,
            tc.tile_pool(name="dram", bufs=1, space="DRAM") as drampool,
            tc.tile_pool(name="xin", bufs=3) as xpool,
            tc.tile_pool(name="xgb", bufs=3) as xgbpool,
            tc.tile_pool(name="trp", bufs=2, space="PSUM") as trpool,
            tc.tile_pool(name="wrm", bufs=1, space="PSUM") as warmpool,
            tc.tile_pool(name="xts", bufs=3) as xtspool,
            tc.tile_pool(name="gt", bufs=1) as gtpool,
            tc.tile_pool(name="sq", bufs=1) as sqpool,
            tc.tile_pool(name="po", bufs=5, space="PSUM") as popool,
            tc.tile_pool(name="osb", bufs=2) as opool,
        ):
            # warm-up source with no DMA dependency (memset on gpsimd)
            warmsrc = persist.tile([128, 512], bf16)
            nc.gpsimd.memset(warmsrc[:], 0.5)
            nc.gpsimd.load_library(library_config.mlp)

            # params: idx first on sync (gather prep depends on it); small
            # tiles before the big weight tiles on scalar
            idx = persist.tile([128, F // 16], i16)
            nc.sync.dma_start(idx[:], idx_d)
            ident = persist.tile([128, 128], bf16)
            nc.scalar.dma_start(ident[:], id_d)
            cstt = persist.tile([128, NBLK], f32)
            nc.scalar.dma_start(cstt[:], cstt_d)

            # x tile loads issued before the weight tiles so x0 isn't stuck
            # behind 1 MiB of weights on the DMA engines
            xtiles = []
            for bt in range(NT):
                xf = xpool.tile([128, F], f32, tag=f"x{bt}")
                eng = nc.sync if bt % 2 == 0 else nc.scalar
                eng.dma_start(xf[:], x_d[bt * 128:(bt + 1) * 128, :])
                xtiles.append(xf)

            w_raw = persist.tile([128, RKCOLS], bf16)
            nc.scalar.dma_start(w_raw[:], wraw_d)
            w_sq = persist.tile([128, RKCOLS], bf16)
            nc.scalar.dma_start(w_sq[:], wsq_d)

            # HBM scratch holding xT (feature-major, bf16): row f = 512 vals
            xt_dram = drampool.tile([F, BL], bf16)
            xt_wview = xt_dram[:].rearrange("(c p) b -> p c b", p=128)

            # gathered tile: [128 p, pair, 512 b]; gathered row i lands at
            # partition i%128, chunk i//128 -> chunk j = pair j's 128 rows
            gt = gtpool.tile([128, NPAIR * BL], bf16)
            gsems = [nc.alloc_semaphore(f"gsem{q}") for q in range(2)]
            gt_view = gt[:].rearrange("p (c b) -> p c b", c=NPAIR)
            xt_rows = xt_dram[:].rearrange("(a f) b -> a f b", a=1)[0]

            # trigger the Scalar ACT table load early (off the critical path)
            actwarm = persist.tile([128, 1], bf16)
            nc.scalar.activation(
                actwarm[:], cstt[:, 0:1],
                mybir.ActivationFunctionType.Identity,
                bias=cstt[:, 0:1],
            )

            warm = warmpool.tile([128, 512], f32)

            def warm_mm(n):
                for _ in range(n):
                    nc.tensor.matmul(
                        warm[:], warmsrc[:, 0:128], warmsrc[:],
                        start=True, stop=True,
                    )

            # ---- phase 1: cast on Scalar, transpose bf16 on PE ----
            warm_mm(N_PRE)
            for bt in range(NT):
                xf = xtiles[bt]
                xgb = xgbpool.tile([128, F], bf16)
                nc.scalar.copy(xgb[:], xf[:])
                xts = xtspool.tile([128, F], bf16)  # [128, 8 chunks, 128 b]
                for half in range(2):
                    pt = trpool.tile([128, 512], bf16)
                    for j in range(4):
                        c = 4 * half + j
                        nc.tensor.transpose(
                            pt[:, j * 128:(j + 1) * 128],
                            xgb[:, c * 128:(c + 1) * 128],
                            ident[:],
                        )
                    nc.vector.tensor_copy(
                        xts[:, half * 512:(half + 1) * 512], pt[:]
                    )
                nc.sync.dma_start(
                    xt_wview[:, :, bt * 128:(bt + 1) * 128],
                    xts[:].rearrange("p (c b) -> p c b", c=8),
                )
                if bt < NT - 1:
                    warm_mm(N_MID)

            # ---- phase 2: two plain 512-idx gathers; gather#0's DMA flies
            # while gather#1's desc-gen runs, and phase 3 starts on pairs 0-3
            # as soon as gather#0 lands ----
            for q in range(2):
                nc.gpsimd.dma_gather(
                    out_ap=gt_view[:, 4 * q:4 * q + 4],
                    in_ap=xt_rows,
                    idxs_ap=idx[:, 32 * q:32 * q + 32],
                    num_idxs=F // 2,
                    num_idxs_reg=F // 2,
                    elem_size=BL,
                    queue_num=q,
                )
            warm_mm(N_PRE3)

            # ---- phase 3: outT block matmuls + fused const-add drain ----
            sq = sqpool.tile([128, NPAIR * BL], bf16)
            osb = None
            for j in range(NPAIR):
                js = slice(j * BL, (j + 1) * BL)
                nc.vector.tensor_mul(sq[:, js], gt[:, js], gt[:, js])
                for h in range(2):
                    g = 2 * j + h
                    slot = g % 4
                    if slot == 0:
                        osb = opool.tile([128, 4 * BL], bf16)
                    po = popool.tile([128, BL], f32)
                    nc.tensor.matmul(
                        po[:], w_raw[:, g * 128:(g + 1) * 128], gt[:, js],
                        start=True, stop=False,
                    )
                    nc.tensor.matmul(
                        po[:], w_sq[:, g * 128:(g + 1) * 128], sq[:, js],
                        start=False, stop=True,
                    )
                    if h == 0:
                        nc.scalar.activation(
                            osb[:, slot * BL:(slot + 1) * BL], po[:],
                            mybir.ActivationFunctionType.Identity,
                            bias=cstt[:, g:g + 1],
                        )
                    else:
                        nc.vector.tensor_scalar_add(
                            osb[:, slot * BL:(slot + 1) * BL], po[:],
                            cstt[:, g:g + 1],
                        )
                    if slot == 3:
                        q = g // 4
                        eng = nc.sync
                        eng.dma_start(
                            out_d[4 * q:4 * q + 4].rearrange("c p b -> p c b"),
                            osb[:].rearrange("p (c b) -> p c b", c=4),
                        )

    nc.compile()
    _module_cache["nc"] = nc
    return nc


def _prep_params(regions, means, scales):
    """Host folding of the small [R,K,D] params into matmul weights."""
    regions = np.asarray(regions).astype(np.int64)
    means = np.asarray(means, dtype=np.float64)
    scales = np.asarray(scales, dtype=np.float64)

    inv2 = 1.0 / scales**2                                   # [R,K,D]
    wsq_c = -0.5 * inv2                                      # coeff of x^2
    wraw_c = means * inv2                                    # coeff of x
    const = (
        -0.5 * np.sum(means**2 * inv2, axis=-1)
        - np.sum(np.log(scales), axis=-1)
        - 0.5 * D * LOG_2PI
    )                                                        # [R,K]

    # Block-diagonal weight tiles: pair p covers regions 8p..8p+7.
    # Row 16j+d (region-local j in 0..7), col 32j+k within the pair's 256.
    wraw = np.zeros((128, RKCOLS), np.float32)
    wsq = np.zeros((128, RKCOLS), np.float32)
    for p in range(NPAIR):
        for j in range(8):
            r = 8 * p + j
            rows = slice(16 * j, 16 * j + 16)
            cols = slice(256 * p + 32 * j, 256 * p + 32 * j + 32)
            wraw[rows, cols] = wraw_c[r].T.astype(np.float32)   # [D, K]
            wsq[rows, cols] = wsq_c[r].T.astype(np.float32)
    wraw = wraw.astype(ml_dtypes.bfloat16)
    wsq = wsq.astype(ml_dtypes.bfloat16)

    # per-partition const bias, one column per rk-block of 128:
    # flat col 32r + k -> block g = col // 128, row = col % 128
    cstt = (
        const.reshape(-1).astype(np.float32).reshape(NBLK, 128).T.copy()
    )                                                        # [128, 16]

    # dma_gather index layout: index j lives at [j % 16, j // 16],
    # replicated across the eight 16-partition groups.
    perm = regions.reshape(-1).astype(np.int16)              # [1024]
    idx16 = perm.reshape(F // 16, 16).T                      # [16, 64]
    idx = np.tile(idx16, (8, 1)).copy()                      # [128, 64]

    ident = np.eye(128, dtype=ml_dtypes.bfloat16)
    return wraw, wsq, cstt, idx, ident


def _run(inputs, trace=False, **kwargs):
    x = np.ascontiguousarray(np.asarray(inputs["x"], dtype=np.float32))
    assert x.shape == (B, F), x.shape
    wraw, wsq, cstt, idx, ident = _prep_params(
        inputs["regions"], inputs["means"], inputs["scales"]
    )

    nc = _build_module()
    in_maps = []
    for c in range(NCORES):
        in_maps.append({
            "x": np.ascontiguousarray(x[c * BL:(c + 1) * BL]),
            "wraw": wraw,
            "wsq": wsq,
            "cstt": cstt,
            "idx": idx,
            "ident": ident,
        })
    res = run_bass_kernel_spmd(
        nc, in_maps, core_ids=list(range(NCORES)), trace=trace, **kwargs
    )
    # reassemble: per core outT [16, 128, 512] bf16 -> [2048 rk, 512 b]
    parts = []
    for c in range(NCORES):
        ot = np.asarray(res.results[c]["out"]).reshape(RKCOLS, BL)
        parts.append(ot.T)                                   # [512 b, 2048 rk]
    out = np.concatenate(parts, axis=0).astype(np.float32).reshape(B, R, K)
    return out, res


def kernel(**inputs):
    out, _ = _run(inputs, trace=False)
    return out
